# revision 38
# baseline (speedup 1.0000x reference)
"""Trainium2 Bass kernel for nn_BimModel (retrieval_knn).

Strategy:
  - Algebraic folding (host, fp64): the final projection W_proj [64,3] applied
    per 64-wide block of y commutes with W_end, so W_end [1536,4608] folds to
    Wfold [4608,72].  All pure-linear chains fold similarly:
        logits = featA @ PM,  PM = WbA @ Wq @ Memory.T / sqrt(1536)   [98,512]
        sim*|h| = featA @ Pm, Pm = WbA @ mn.T                          [98,512]
        |h|^2   = rowsum((featA @ G) * featA), G = WbA @ WbA.T         [98,98]
        raw     = att @ MA + w_ep @ EC + featA @ FB (+ biases folded)
    featA = [scaled(96), log(scale), 1.0] (98-dim; the always-zero loc feature
    is dropped; biases ride on the const-1 column).
  - Top-8 episodic retrieval: done post-exp with nc.vector.max (one-instruction
    top-8 per partition); the gather becomes a masked-softmax matmul.
  - Data parallel over 8 NeuronCores: 1024 batch rows each; folded weights
    (~0.6 MB) replicated.
  - The attention-logits matmul runs in float32r (fast PE fp32 mode, ~1.6e-4
    rel err) — safe because softmax is smooth.  The similarity matmul (Pm)
    stays full fp32: top-8 SELECTION is discrete and sensitive to ties.
  - Mean-scaler + per-row scalars are batched [128, 8] across the 8 row-tiles;
    SBUF-only elementwise ops are offloaded to the otherwise-idle GPSIMD.
"""

import numpy as np

import concourse.bacc as bacc
import concourse.mybir as mybir
from concourse.tile import TileContext
from concourse.bass_utils import run_bass_kernel_spmd

N_CORES = 8
B_TOTAL = 8192
ROWS = B_TOTAL // N_CORES          # 1024 rows per core
N_TILES = ROWS // 128              # 8 tiles of 128 rows
C_IN = 96
MEM = 512
KF = 98                            # folded feature dim (scaled96 + logscale + one)
KP = 128                           # padded contraction dim
OUTW = 72                          # 24 preds x 3 params
PRED_LEN = 24
E2 = float(np.exp(2.0))            # ln(E2*x + E2) = softplus(ln x) + 2 trick

FP32 = mybir.dt.float32
F32R = mybir.dt.float32r
AX = mybir.AxisListType.X
OP = mybir.AluOpType
AF = mybir.ActivationFunctionType

_PROGRAM_CACHE: dict = {}


def _fold_weights(W_backbone, b_backbone, Wq, Memory, episodic_memory,
                  W_end, b_end, W_proj, b_proj):
    f64 = np.float64
    Wb = W_backbone.astype(f64)
    bb = b_backbone.astype(f64)
    Wqd = Wq.astype(f64)
    M = Memory.astype(f64)
    E = episodic_memory.astype(f64)
    We = W_end.astype(f64)
    be = b_end.astype(f64)
    Wp = W_proj.astype(f64)
    bp = b_proj.astype(f64)

    # 98-dim augmented backbone (drop always-zero loc feature, add bias row)
    WbA = np.concatenate([Wb[0:96], Wb[97:98], bb[None, :]], axis=0)     # [98,1536]

    Wfold = (We.T.reshape(4608, PRED_LEN, 64) @ Wp).reshape(4608, OUTW)  # [4608,72]
    bfold = (be.reshape(PRED_LEN, 64) @ Wp + bp).reshape(OUTW)           # [72]
    WfA, WfB, WfC = Wfold[0:1536], Wfold[1536:3072], Wfold[3072:4608]

    PM = WbA @ Wqd @ M.T / np.sqrt(f64(1536))                            # [98,512]
    En = E / np.clip(np.linalg.norm(E, axis=-1, keepdims=True), 1e-6, None)
    Pm = WbA @ En.T                                                      # [98,512]
    G = WbA @ WbA.T                                                      # [98,98]
    MA = M @ WfA                                                         # [512,72]
    EC = E @ WfC                                                         # [512,72]
    FB = WbA @ WfB                                                       # [98,72]
    FB[97] += bfold

    def pad_k(a):  # pad leading (contraction) dim 98 -> 128 with zeros
        out = np.zeros((KP, a.shape[1]), np.float32)
        out[: a.shape[0]] = a
        return out

    def chunked(a):  # [512,72] -> [128, 4*72] with chunk c at cols [72c:72c+72]
        return np.ascontiguousarray(
            a.reshape(4, 128, OUTW).transpose(1, 0, 2).reshape(128, 4 * OUTW),
            np.float32)

    GFB = np.concatenate([G, FB], axis=1)                                # [98,170]
    return {
        "PM": pad_k(PM),
        "Pm": pad_k(Pm),
        "GFB": pad_k(GFB),
        "MA": chunked(MA),
        "EC": chunked(EC),
        "ident": np.eye(128, dtype=np.float32),
    }


def _build_program(gp_offload=True, ft_own_pool=True, psl=1, psh=1, pst=3, wpb=4, spb=4):
    nc = bacc.Bacc()

    xw_d = nc.dram_tensor("xw", [ROWS, 2 * C_IN], FP32, kind="ExternalInput")
    pm_d = nc.dram_tensor("PM", [KP, MEM], F32R, kind="ExternalInput")
    pmn_d = nc.dram_tensor("Pm", [KP, MEM], FP32, kind="ExternalInput")
    gfb_d = nc.dram_tensor("GFB", [KP, KF + OUTW], FP32, kind="ExternalInput")
    ma_d = nc.dram_tensor("MA", [128, 4 * OUTW], FP32, kind="ExternalInput")
    ec_d = nc.dram_tensor("EC", [128, 4 * OUTW], FP32, kind="ExternalInput")
    id_d = nc.dram_tensor("ident", [128, 128], FP32, kind="ExternalInput")

    raw_d = nc.dram_tensor("raw", [ROWS, OUTW], FP32, kind="ExternalOutput")
    scl_d = nc.dram_tensor("scl", [128, N_TILES], FP32, kind="ExternalOutput")

    with TileContext(nc) as tc:
        with (
            tc.tile_pool(name="consts", bufs=1) as cpool,
            tc.tile_pool(name="resid", bufs=1) as rp,
            tc.tile_pool(name="work", bufs=wpb) as wp,
            tc.tile_pool(name="small", bufs=spb) as sp,
            tc.tile_pool(name="psL", bufs=psl, space="PSUM") as psL,       # logits
            tc.tile_pool(name="psH", bufs=psh, space="PSUM") as psH,                           # hm
            tc.tile_pool(name="psG", bufs=1, space="PSUM") as psG,       # gfb
            tc.tile_pool(name="psT", bufs=pst, space="PSUM") as psT,       # transposes
            tc.tile_pool(name="psO", bufs=1, space="PSUM") as psO,       # out72
            tc.tile_pool(name="psF", bufs=1, space="PSUM") as psF,       # featT
        ):
            ft_pool = psF if ft_own_pool else psT
            ft_tag = "fT" if ft_own_pool else "tr"
            gv = nc.gpsimd if gp_offload else nc.vector
            pm_sb = cpool.tile_from(pm_d[:])
            pmn_sb = cpool.tile_from(pmn_d[:])
            gfb_sb = cpool.tile_from(gfb_d[:])
            ma_sb = cpool.tile_from(ma_d[:])
            ec_sb = cpool.tile_from(ec_d[:])
            id_sb = cpool.tile_from(id_d[:])
            e2c = cpool.tile([128, 1], FP32, tag="e2c")
            nc.vector.memset(e2c[:], E2)

            # Pin the ACT table to natural_log_exp_and_others (covers Exp, Ln,
            # Copy, Abs) so bacc's per-function chooser doesn't thrash between
            # exp_and_others and natural_log (45 reloads ~= 58us otherwise).
            nc.scalar.add_instruction(mybir.InstLoadActFuncSet(
                name=nc.get_next_instruction_name(), act_func_set_id=6,
                ins=[], outs=[]))

            # ---- load all row-tiles; batched mean-scaler over [128, 8] ----
            xin = rp.tile([128, N_TILES * 2 * C_IN], FP32, tag="xin")
            for t in range(N_TILES):
                nc.sync.dma_start(out=xin[:, t * 192:(t + 1) * 192],
                                  in_=xw_d[t * 128:(t + 1) * 128, :])
            xin3 = xin[:].rearrange("p (t c) -> p t c", c=192)
            xabs = rp.tile([128, N_TILES * C_IN], FP32, tag="xabs")
            xabs3 = xabs[:].rearrange("p (t c) -> p t c", c=C_IN)
            nc.vector.tensor_tensor(out=xabs3, in0=xin3[:, :, 0:C_IN],
                                    in1=xin3[:, :, C_IN:192], op=OP.mult)
            ts8 = rp.tile([128, N_TILES], FP32, tag="ts8")
            nc.vector.tensor_reduce(out=ts8[:], in_=xabs3, axis=AX, op=OP.add,
                                    apply_absolute_value=True)
            nobs8 = rp.tile([128, N_TILES], FP32, tag="nobs8")
            nc.vector.tensor_reduce(out=nobs8[:], in_=xin3[:, :, C_IN:192],
                                    axis=AX, op=OP.add)
            gv.tensor_scalar_max(nobs8[:], nobs8[:], 1.0)
            rn8 = rp.tile([128, N_TILES], FP32, tag="rn8")
            nc.vector.reciprocal(out=rn8[:], in_=nobs8[:])
            scale8 = rp.tile([128, N_TILES], FP32, tag="scale8")
            nc.vector.tensor_tensor(out=scale8[:], in0=ts8[:], in1=rn8[:],
                                    op=OP.mult)
            nc.vector.tensor_scalar_max(scale8[:], scale8[:], 1e-10)
            nc.sync.dma_start(out=scl_d[:], in_=scale8[:])
            logscale8 = rp.tile([128, N_TILES], FP32, tag="logscale8")
            nc.scalar.activation(out=logscale8[:], in_=scale8[:], func=AF.Ln)
            rs8 = rp.tile([128, N_TILES], FP32, tag="rs8")
            nc.vector.reciprocal(out=rs8[:], in_=scale8[:])

            for t in range(N_TILES):
                rs0 = t * 128
                x_sb = xin[:, t * 192:t * 192 + C_IN]

                # ---- featA = [x/scale, ln(scale), 1.0] ----
                featA = wp.tile([128, KF], FP32, tag="featA")
                nc.vector.tensor_scalar(out=featA[:, 0:C_IN], in0=x_sb,
                                        scalar1=rs8[:, t:t + 1], scalar2=None,
                                        op0=OP.mult)
                gv.tensor_copy(out=featA[:, 96:97],
                                      in_=logscale8[:, t:t + 1])
                gv.memset(featA[:, 97:98], 1.0)

                # ---- transpose featA -> featT[0:98]; pad rows are garbage,
                #      harmless: weight rows 98:127 are zero ----
                fT_ps = ft_pool.tile([128, 128], FP32, tag=ft_tag)
                nc.tensor.transpose(fT_ps[0:KF, :], featA[:], id_sb[:])
                featT = wp.tile([128, 128], FP32, tag="featT")
                gv.memset(featT[96:128, :], 0.0)
                nc.scalar.copy(out=featT[0:KF, :], in_=fT_ps[0:KF, :])
                featTr = wp.tile([128, 128], F32R, tag="featTr")
                nc.vector.tensor_copy(out=featTr[0:KF, :], in_=fT_ps[0:KF, :])

                # ---- stage-1 matmuls (featT stationary) ----
                logits_ps = psL.tile([128, MEM], FP32, tag="logits")
                hm_ps = psH.tile([128, MEM], FP32, tag="hm")
                gfb_ps = psG.tile([128, KF], FP32, tag="gfbo")
                nc.tensor.matmul(logits_ps[:], featTr[0:KF, :], pm_sb[0:KF, :],
                                 start=True, stop=True)
                nc.tensor.matmul(hm_ps[:], featT[:], pmn_sb[:], start=True,
                                 stop=True)
                nc.tensor.matmul(gfb_ps[:], featT[:], gfb_sb[:, 0:KF],
                                 start=True, stop=True)

                # ---- attention softmax (no max-sub; logits bounded) ----
                e_att = wp.tile([128, MEM], FP32, tag="e_att")
                datt = sp.tile([128, 1], FP32, tag="datt")
                nc.scalar.activation(out=e_att[:], in_=logits_ps[:], func=AF.Exp,
                                     accum_out=datt[:])
                rdatt = sp.tile([128, 1], FP32, tag="rdatt")
                nc.vector.reciprocal(out=rdatt[:], in_=datt[:])
                e_attn = wp.tile([128, MEM], FP32, tag="e_attn")
                gv.tensor_scalar(out=e_attn[:], in0=e_att[:],
                                        scalar1=rdatt[:], scalar2=None,
                                        op0=OP.mult)

                # ---- 1/||h|| via featA G featA^T; rsqrt = exp(-0.5 ln) ----
                hgf = wp.tile([128, KF], FP32, tag="hgf")
                nc.vector.tensor_tensor(out=hgf[:], in0=gfb_ps[:, 0:KF],
                                        in1=featA[:], op=OP.mult)
                normsq = sp.tile([128, 1], FP32, tag="normsq")
                nc.vector.tensor_reduce(out=normsq[:], in_=hgf[:], axis=AX,
                                        op=OP.add)
                nc.vector.tensor_scalar_max(normsq[:], normsq[:], 1e-12)
                lnq = sp.tile([128, 1], FP32, tag="lnq")
                nc.scalar.activation(out=lnq[:], in_=normsq[:], func=AF.Ln)
                r_sb = sp.tile([128, 1], FP32, tag="r")
                nc.scalar.activation(out=r_sb[:], in_=lnq[:], func=AF.Exp,
                                     scale=-0.5)

                # ---- episodic: top-8 of cos post-exp, masked softmax ----
                e_cos = wp.tile([128, MEM], FP32, tag="e_cos")
                nc.scalar.activation(out=e_cos[:], in_=hm_ps[:], func=AF.Exp,
                                     scale=r_sb[:])
                m8 = sp.tile([128, 8], FP32, tag="m8")
                nc.vector.max(out=m8[:], in_=e_cos[:])
                dep = sp.tile([128, 1], FP32, tag="dep")
                nc.vector.tensor_reduce(out=dep[:], in_=m8[:], axis=AX, op=OP.add)
                rdep = sp.tile([128, 1], FP32, tag="rdep")
                nc.vector.reciprocal(out=rdep[:], in_=dep[:])
                mask_s = wp.tile([128, MEM], FP32, tag="mask")
                gv.tensor_scalar(out=mask_s[:], in0=e_cos[:],
                                        scalar1=m8[:, 7:8], scalar2=rdep[:],
                                        op0=OP.is_ge, op1=OP.mult)
                e_ep = wp.tile([128, MEM], FP32, tag="e_ep")
                nc.vector.tensor_tensor(out=e_ep[:], in0=e_cos[:], in1=mask_s[:],
                                        op=OP.mult)

                # ---- transpose e_attn / e_ep via PE, copy PSUM->SBUF ----
                eT = wp.tile([128, MEM], FP32, tag="eT")
                epT = wp.tile([128, MEM], FP32, tag="epT")
                for c in range(4):
                    sl = slice(c * 128, (c + 1) * 128)
                    tr1 = psT.tile([128, 128], FP32, tag="tr")
                    nc.tensor.transpose(tr1[:], e_attn[:, sl], id_sb[:])
                    if c % 2 == 0:
                        nc.vector.tensor_copy(out=eT[:, sl], in_=tr1[:])
                    else:
                        nc.scalar.copy(out=eT[:, sl], in_=tr1[:])
                    tr2 = psT.tile([128, 128], FP32, tag="tr")
                    nc.tensor.transpose(tr2[:], e_ep[:, sl], id_sb[:])
                    if c % 2 == 0:
                        nc.scalar.copy(out=epT[:, sl], in_=tr2[:])
                    else:
                        nc.vector.tensor_copy(out=epT[:, sl], in_=tr2[:])

                # ---- stage-2: raw72 = feat@FB + att@MA + ep@EC (PSUM accum).
                #      Tile pairs share one [128,144] PSUM tile so the head
                #      runs once per pair (param stride 3 is uniform across
                #      both 72-col halves). ----
                if t % 2 == 0:
                    o72_pair = psO.tile([128, 2 * OUTW], FP32, tag="o72")
                o72_ps = o72_pair[:, (t % 2) * OUTW:(t % 2 + 1) * OUTW]
                nc.tensor.matmul(o72_ps, featT[:], gfb_sb[:, KF:KF + OUTW],
                                 start=True, stop=False)
                for c in range(4):
                    sl = slice(c * 128, (c + 1) * 128)
                    cw = slice(c * OUTW, (c + 1) * OUTW)
                    nc.tensor.matmul(o72_ps, eT[:, sl], ma_sb[:, cw],
                                     start=False, stop=False)
                for c in range(4):
                    sl = slice(c * 128, (c + 1) * 128)
                    cw = slice(c * OUTW, (c + 1) * OUTW)
                    nc.tensor.matmul(o72_ps, epT[:, sl], ec_sb[:, cw],
                                     start=False, stop=(c == 3))
                if t % 2 == 0:
                    continue

                # ---- StudentT head for the pair (reads [128,144] PSUM) ----
                out_sb = wp.tile([128, 2 * OUTW], FP32, tag="out")
                t48 = wp.tile([128, 2 * PRED_LEN], FP32, tag="t48")
                # df = 2 + softplus(raw0) = ln(E2*exp(raw0) + E2)
                nc.scalar.activation(out=t48[:], in_=o72_pair[:, 0::3], func=AF.Exp)
                nc.scalar.activation(out=out_sb[:, 0::3], in_=t48[:], func=AF.Ln,
                                     scale=E2, bias=e2c[:])
                nc.vector.tensor_copy(out=out_sb[:, 1::3], in_=o72_pair[:, 1::3])
                t48b = wp.tile([128, 2 * PRED_LEN], FP32, tag="t48b")
                nc.scalar.activation(out=t48b[:], in_=o72_pair[:, 2::3], func=AF.Exp)
                nc.scalar.activation(out=out_sb[:, 2::3], in_=t48b[:], func=AF.Ln,
                                     bias=1.0)
                nc.sync.dma_start(out=raw_d[rs0 - 128:rs0, :],
                                  in_=out_sb[:, 0:OUTW])
                nc.sync.dma_start(out=raw_d[rs0:rs0 + 128, :],
                                  in_=out_sb[:, OUTW:2 * OUTW])

    return nc


def kernel(past_target, past_observed_values, W_backbone, b_backbone, Wq,
           Memory, episodic_memory, W_end, b_end, W_proj, b_proj):
    past_target = np.ascontiguousarray(past_target, np.float32)
    past_observed_values = np.ascontiguousarray(past_observed_values, np.float32)

    consts = _fold_weights(W_backbone, b_backbone, Wq, Memory, episodic_memory,
                           W_end, b_end, W_proj, b_proj)

    if "nc" not in _PROGRAM_CACHE:
        nc = _build_program()
        nc.finalize()
        _PROGRAM_CACHE["nc"] = nc
    nc = _PROGRAM_CACHE["nc"]

    in_maps = []
    for c in range(N_CORES):
        sl = slice(c * ROWS, (c + 1) * ROWS)
        m = {"xw": np.ascontiguousarray(
            np.concatenate([past_target[sl], past_observed_values[sl]], axis=1))}
        m.update(consts)
        in_maps.append(m)

    res = run_bass_kernel_spmd(nc, in_maps, list(range(N_CORES)))
    globals()["_LAST_RESULTS"] = res
    raw = np.concatenate([r["raw"] for r in res.results], axis=0)       # [B,72]
    # scl is [128, 8] per core with scale[p, t] = row t*128+p
    scale = np.concatenate(
        [r["scl"].T.reshape(ROWS, 1) for r in res.results], axis=0)     # [B,1]

    distr = raw.reshape(B_TOTAL, PRED_LEN, 3)
    loc = np.zeros_like(scale)
    return distr, loc, scale


# revision 41
# speedup vs baseline: 1.0335x; 1.0335x over previous
"""Trainium2 Bass kernel for nn_BimModel (retrieval_knn).

Strategy:
  - Algebraic folding (host, fp64): the final projection W_proj [64,3] applied
    per 64-wide block of y commutes with W_end, so W_end [1536,4608] folds to
    Wfold [4608,72].  All pure-linear chains fold similarly:
        logits = featA @ PM,  PM = WbA @ Wq @ Memory.T / sqrt(1536)   [98,512]
        sim*|h| = featA @ Pm, Pm = WbA @ mn.T                          [98,512]
        |h|^2   = rowsum((featA @ G) * featA), G = WbA @ WbA.T         [98,98]
        raw     = att @ MA + w_ep @ EC + featA @ FB (+ biases folded)
    featA = [scaled(96), log(scale), 1.0] (98-dim; the always-zero loc feature
    is dropped; biases ride on the const-1 column).
  - Top-8 episodic retrieval: done post-exp with nc.vector.max (one-instruction
    top-8 per partition); the gather becomes a masked-softmax matmul.
  - Data parallel over 8 NeuronCores: 1024 batch rows each; folded weights
    (~0.6 MB) replicated.
  - The attention-logits matmul runs in float32r (fast PE fp32 mode, ~1.6e-4
    rel err) — safe because softmax is smooth.  The similarity matmul (Pm)
    stays full fp32: top-8 SELECTION is discrete and sensitive to ties.
  - Mean-scaler + per-row scalars are batched [128, 8] across the 8 row-tiles;
    SBUF-only elementwise ops are offloaded to the otherwise-idle GPSIMD.
"""

import numpy as np

import concourse.bacc as bacc
import concourse.mybir as mybir
from concourse.tile import TileContext
from concourse.bass_utils import run_bass_kernel_spmd

N_CORES = 8
B_TOTAL = 8192
ROWS = B_TOTAL // N_CORES          # 1024 rows per core
N_TILES = ROWS // 128              # 8 tiles of 128 rows
C_IN = 96
MEM = 512
KF = 98                            # folded feature dim (scaled96 + logscale + one)
KP = 128                           # padded contraction dim
OUTW = 72                          # 24 preds x 3 params
PRED_LEN = 24
E2 = float(np.exp(2.0))            # ln(E2*x + E2) = softplus(ln x) + 2 trick

FP32 = mybir.dt.float32
F32R = mybir.dt.float32r
AX = mybir.AxisListType.X
OP = mybir.AluOpType
AF = mybir.ActivationFunctionType

_PROGRAM_CACHE: dict = {}


def _fold_weights(W_backbone, b_backbone, Wq, Memory, episodic_memory,
                  W_end, b_end, W_proj, b_proj):
    f64 = np.float64
    Wb = W_backbone.astype(f64)
    bb = b_backbone.astype(f64)
    Wqd = Wq.astype(f64)
    M = Memory.astype(f64)
    E = episodic_memory.astype(f64)
    We = W_end.astype(f64)
    be = b_end.astype(f64)
    Wp = W_proj.astype(f64)
    bp = b_proj.astype(f64)

    # 98-dim augmented backbone (drop always-zero loc feature, add bias row)
    WbA = np.concatenate([Wb[0:96], Wb[97:98], bb[None, :]], axis=0)     # [98,1536]

    Wfold = (We.T.reshape(4608, PRED_LEN, 64) @ Wp).reshape(4608, OUTW)  # [4608,72]
    bfold = (be.reshape(PRED_LEN, 64) @ Wp + bp).reshape(OUTW)           # [72]
    WfA, WfB, WfC = Wfold[0:1536], Wfold[1536:3072], Wfold[3072:4608]

    PM = WbA @ Wqd @ M.T / np.sqrt(f64(1536))                            # [98,512]
    En = E / np.clip(np.linalg.norm(E, axis=-1, keepdims=True), 1e-6, None)
    Pm = WbA @ En.T                                                      # [98,512]
    G = WbA @ WbA.T                                                      # [98,98]
    MA = M @ WfA                                                         # [512,72]
    EC = E @ WfC                                                         # [512,72]
    FB = WbA @ WfB                                                       # [98,72]
    FB[97] += bfold

    def pad_k(a):  # pad leading (contraction) dim 98 -> 128 with zeros
        out = np.zeros((KP, a.shape[1]), np.float32)
        out[: a.shape[0]] = a
        return out

    def chunked(a):  # [512,72] -> [128, 4*72] with chunk c at cols [72c:72c+72]
        return np.ascontiguousarray(
            a.reshape(4, 128, OUTW).transpose(1, 0, 2).reshape(128, 4 * OUTW),
            np.float32)

    GFB = np.concatenate([G, FB], axis=1)                                # [98,170]
    return {
        "PM": pad_k(PM),
        "Pm": pad_k(Pm),
        "GFB": pad_k(GFB),
        "MA": chunked(MA),
        "EC": chunked(EC),
        "ident": np.eye(128, dtype=np.float32),
    }


def _build_program(gp_offload=True, ft_own_pool=True, psl=1, psh=1, pst=3, wpb=4, spb=4):
    nc = bacc.Bacc()

    xw_d = nc.dram_tensor("xw", [ROWS, 2 * C_IN], FP32, kind="ExternalInput")
    pm_d = nc.dram_tensor("PM", [KP, MEM], F32R, kind="ExternalInput")
    pmn_d = nc.dram_tensor("Pm", [KP, MEM], FP32, kind="ExternalInput")
    gfb_d = nc.dram_tensor("GFB", [KP, KF + OUTW], FP32, kind="ExternalInput")
    ma_d = nc.dram_tensor("MA", [128, 4 * OUTW], FP32, kind="ExternalInput")
    ec_d = nc.dram_tensor("EC", [128, 4 * OUTW], FP32, kind="ExternalInput")
    id_d = nc.dram_tensor("ident", [128, 128], FP32, kind="ExternalInput")

    raw_d = nc.dram_tensor("raw", [ROWS, OUTW], FP32, kind="ExternalOutput")
    scl_d = nc.dram_tensor("scl", [128, N_TILES], FP32, kind="ExternalOutput")

    with TileContext(nc) as tc:
        with (
            tc.tile_pool(name="consts", bufs=1) as cpool,
            tc.tile_pool(name="resid", bufs=1) as rp,
            tc.tile_pool(name="work", bufs=wpb) as wp,
            tc.tile_pool(name="small", bufs=spb) as sp,
            tc.tile_pool(name="psL", bufs=psl, space="PSUM") as psL,       # logits
            tc.tile_pool(name="psH", bufs=psh, space="PSUM") as psH,                           # hm
            tc.tile_pool(name="psG", bufs=1, space="PSUM") as psG,       # gfb
            tc.tile_pool(name="psT", bufs=pst, space="PSUM") as psT,       # transposes
            tc.tile_pool(name="psO", bufs=1, space="PSUM") as psO,       # out72
            tc.tile_pool(name="psF", bufs=1, space="PSUM") as psF,       # featT
        ):
            ft_pool = psF if ft_own_pool else psT
            ft_tag = "fT" if ft_own_pool else "tr"
            gv = nc.gpsimd if gp_offload else nc.vector
            pm_sb = cpool.tile_from(pm_d[:])
            pmn_sb = cpool.tile_from(pmn_d[:])
            gfb_sb = cpool.tile_from(gfb_d[:])
            ma_sb = cpool.tile_from(ma_d[:])
            ec_sb = cpool.tile_from(ec_d[:])
            id_sb = cpool.tile_from(id_d[:])
            e2c = cpool.tile([128, 1], FP32, tag="e2c")
            nc.vector.memset(e2c[:], E2)

            # Pin the ACT table to natural_log_exp_and_others (covers Exp, Ln,
            # Copy, Abs) so bacc's per-function chooser doesn't thrash between
            # exp_and_others and natural_log (45 reloads ~= 58us otherwise).
            nc.scalar.add_instruction(mybir.InstLoadActFuncSet(
                name=nc.get_next_instruction_name(), act_func_set_id=6,
                ins=[], outs=[]))

            # ---- load all row-tiles; batched mean-scaler over [128, 8] ----
            xin = rp.tile([128, N_TILES * 2 * C_IN], FP32, tag="xin")
            xin3 = xin[:].rearrange("p (t c) -> p t c", c=192)
            nc.sync.dma_start(out=xin3,
                              in_=xw_d[:].rearrange("(t p) c -> p t c", p=128))
            xabs = rp.tile([128, N_TILES * C_IN], FP32, tag="xabs")
            xabs3 = xabs[:].rearrange("p (t c) -> p t c", c=C_IN)
            nc.vector.tensor_tensor(out=xabs3, in0=xin3[:, :, 0:C_IN],
                                    in1=xin3[:, :, C_IN:192], op=OP.mult)
            ts8 = rp.tile([128, N_TILES], FP32, tag="ts8")
            nc.vector.tensor_reduce(out=ts8[:], in_=xabs3, axis=AX, op=OP.add,
                                    apply_absolute_value=True)
            nobs8 = rp.tile([128, N_TILES], FP32, tag="nobs8")
            nc.vector.tensor_reduce(out=nobs8[:], in_=xin3[:, :, C_IN:192],
                                    axis=AX, op=OP.add)
            gv.tensor_scalar_max(nobs8[:], nobs8[:], 1.0)
            rn8 = rp.tile([128, N_TILES], FP32, tag="rn8")
            nc.vector.reciprocal(out=rn8[:], in_=nobs8[:])
            scale8 = rp.tile([128, N_TILES], FP32, tag="scale8")
            nc.vector.tensor_tensor(out=scale8[:], in0=ts8[:], in1=rn8[:],
                                    op=OP.mult)
            nc.vector.tensor_scalar_max(scale8[:], scale8[:], 1e-10)
            nc.sync.dma_start(out=scl_d[:], in_=scale8[:])
            logscale8 = rp.tile([128, N_TILES], FP32, tag="logscale8")
            nc.scalar.activation(out=logscale8[:], in_=scale8[:], func=AF.Ln)
            rs8 = rp.tile([128, N_TILES], FP32, tag="rs8")
            nc.vector.reciprocal(out=rs8[:], in_=scale8[:])

            for t in range(N_TILES):
                rs0 = t * 128
                x_sb = xin[:, t * 192:t * 192 + C_IN]

                # ---- featA = [x/scale, ln(scale), 1.0] ----
                featA = wp.tile([128, KF], FP32, tag="featA")
                nc.vector.tensor_scalar(out=featA[:, 0:C_IN], in0=x_sb,
                                        scalar1=rs8[:, t:t + 1], scalar2=None,
                                        op0=OP.mult)
                gv.tensor_copy(out=featA[:, 96:97],
                                      in_=logscale8[:, t:t + 1])
                gv.memset(featA[:, 97:98], 1.0)

                # ---- transpose featA -> featT[0:98]; pad rows are garbage,
                #      harmless: weight rows 98:127 are zero ----
                fT_ps = ft_pool.tile([128, 128], FP32, tag=ft_tag)
                nc.tensor.transpose(fT_ps[0:KF, :], featA[:], id_sb[:])
                featT = wp.tile([128, 128], FP32, tag="featT")
                gv.memset(featT[96:128, :], 0.0)
                nc.scalar.copy(out=featT[0:KF, :], in_=fT_ps[0:KF, :])
                featTr = wp.tile([128, 128], F32R, tag="featTr")
                nc.vector.tensor_copy(out=featTr[0:KF, :], in_=fT_ps[0:KF, :])

                # ---- stage-1 matmuls (featT stationary) ----
                logits_ps = psL.tile([128, MEM], FP32, tag="logits")
                hm_ps = psH.tile([128, MEM], FP32, tag="hm")
                gfb_ps = psG.tile([128, KF], FP32, tag="gfbo")
                nc.tensor.matmul(logits_ps[:], featTr[0:KF, :], pm_sb[0:KF, :],
                                 start=True, stop=True)
                nc.tensor.matmul(hm_ps[:], featT[:], pmn_sb[:], start=True,
                                 stop=True)
                nc.tensor.matmul(gfb_ps[:], featT[:], gfb_sb[:, 0:KF],
                                 start=True, stop=True)

                # ---- attention softmax (no max-sub; logits bounded) ----
                e_att = wp.tile([128, MEM], FP32, tag="e_att")
                datt = sp.tile([128, 1], FP32, tag="datt")
                nc.scalar.activation(out=e_att[:], in_=logits_ps[:], func=AF.Exp,
                                     accum_out=datt[:])
                rdatt = sp.tile([128, 1], FP32, tag="rdatt")
                nc.vector.reciprocal(out=rdatt[:], in_=datt[:])
                e_attn = wp.tile([128, MEM], FP32, tag="e_attn")
                gv.tensor_scalar(out=e_attn[:], in0=e_att[:],
                                        scalar1=rdatt[:], scalar2=None,
                                        op0=OP.mult)

                # ---- 1/||h|| via featA G featA^T; rsqrt = exp(-0.5 ln) ----
                hgf = wp.tile([128, KF], FP32, tag="hgf")
                nc.vector.tensor_tensor(out=hgf[:], in0=gfb_ps[:, 0:KF],
                                        in1=featA[:], op=OP.mult)
                normsq = sp.tile([128, 1], FP32, tag="normsq")
                nc.vector.tensor_reduce(out=normsq[:], in_=hgf[:], axis=AX,
                                        op=OP.add)
                nc.vector.tensor_scalar_max(normsq[:], normsq[:], 1e-12)
                lnq = sp.tile([128, 1], FP32, tag="lnq")
                nc.scalar.activation(out=lnq[:], in_=normsq[:], func=AF.Ln)
                r_sb = sp.tile([128, 1], FP32, tag="r")
                nc.scalar.activation(out=r_sb[:], in_=lnq[:], func=AF.Exp,
                                     scale=-0.5)

                # ---- episodic: top-8 of cos post-exp, masked softmax ----
                e_cos = wp.tile([128, MEM], FP32, tag="e_cos")
                nc.scalar.activation(out=e_cos[:], in_=hm_ps[:], func=AF.Exp,
                                     scale=r_sb[:])
                m8 = sp.tile([128, 8], FP32, tag="m8")
                nc.vector.max(out=m8[:], in_=e_cos[:])
                dep = sp.tile([128, 1], FP32, tag="dep")
                nc.vector.tensor_reduce(out=dep[:], in_=m8[:], axis=AX, op=OP.add)
                rdep = sp.tile([128, 1], FP32, tag="rdep")
                nc.vector.reciprocal(out=rdep[:], in_=dep[:])
                mask_s = wp.tile([128, MEM], FP32, tag="mask")
                gv.tensor_scalar(out=mask_s[:], in0=e_cos[:],
                                        scalar1=m8[:, 7:8], scalar2=rdep[:],
                                        op0=OP.is_ge, op1=OP.mult)
                e_ep = wp.tile([128, MEM], FP32, tag="e_ep")
                nc.vector.tensor_tensor(out=e_ep[:], in0=e_cos[:], in1=mask_s[:],
                                        op=OP.mult)

                # ---- transpose e_attn / e_ep via PE, copy PSUM->SBUF ----
                eT = wp.tile([128, MEM], FP32, tag="eT")
                epT = wp.tile([128, MEM], FP32, tag="epT")
                for c in range(4):
                    sl = slice(c * 128, (c + 1) * 128)
                    tr1 = psT.tile([128, 128], FP32, tag="tr")
                    nc.tensor.transpose(tr1[:], e_attn[:, sl], id_sb[:])
                    if c % 2 == 0:
                        nc.vector.tensor_copy(out=eT[:, sl], in_=tr1[:])
                    else:
                        nc.scalar.copy(out=eT[:, sl], in_=tr1[:])
                    tr2 = psT.tile([128, 128], FP32, tag="tr")
                    nc.tensor.transpose(tr2[:], e_ep[:, sl], id_sb[:])
                    if c % 2 == 0:
                        nc.scalar.copy(out=epT[:, sl], in_=tr2[:])
                    else:
                        nc.vector.tensor_copy(out=epT[:, sl], in_=tr2[:])

                # ---- stage-2: raw72 = feat@FB + att@MA + ep@EC (PSUM accum).
                #      Tile pairs share one [128,144] PSUM tile so the head
                #      runs once per pair (param stride 3 is uniform across
                #      both 72-col halves). ----
                if t % 2 == 0:
                    o72_pair = psO.tile([128, 2 * OUTW], FP32, tag="o72")
                o72_ps = o72_pair[:, (t % 2) * OUTW:(t % 2 + 1) * OUTW]
                nc.tensor.matmul(o72_ps, featT[:], gfb_sb[:, KF:KF + OUTW],
                                 start=True, stop=False)
                for c in range(4):
                    sl = slice(c * 128, (c + 1) * 128)
                    cw = slice(c * OUTW, (c + 1) * OUTW)
                    nc.tensor.matmul(o72_ps, eT[:, sl], ma_sb[:, cw],
                                     start=False, stop=False)
                for c in range(4):
                    sl = slice(c * 128, (c + 1) * 128)
                    cw = slice(c * OUTW, (c + 1) * OUTW)
                    nc.tensor.matmul(o72_ps, epT[:, sl], ec_sb[:, cw],
                                     start=False, stop=(c == 3))
                if t % 2 == 0:
                    continue

                # ---- StudentT head for the pair (reads [128,144] PSUM) ----
                out_sb = wp.tile([128, 2 * OUTW], FP32, tag="out")
                t48 = wp.tile([128, 2 * PRED_LEN], FP32, tag="t48")
                # df = 2 + softplus(raw0) = ln(E2*exp(raw0) + E2)
                nc.scalar.activation(out=t48[:], in_=o72_pair[:, 0::3], func=AF.Exp)
                nc.scalar.activation(out=out_sb[:, 0::3], in_=t48[:], func=AF.Ln,
                                     scale=E2, bias=e2c[:])
                nc.vector.tensor_copy(out=out_sb[:, 1::3], in_=o72_pair[:, 1::3])
                t48b = wp.tile([128, 2 * PRED_LEN], FP32, tag="t48b")
                nc.scalar.activation(out=t48b[:], in_=o72_pair[:, 2::3], func=AF.Exp)
                nc.scalar.activation(out=out_sb[:, 2::3], in_=t48b[:], func=AF.Ln,
                                     bias=1.0)
                nc.sync.dma_start(
                    out=raw_d[rs0 - 128:rs0 + 128, :].rearrange(
                        "(u p) w -> p u w", p=128),
                    in_=out_sb[:].rearrange("p (u w) -> p u w", w=OUTW))

    return nc


def kernel(past_target, past_observed_values, W_backbone, b_backbone, Wq,
           Memory, episodic_memory, W_end, b_end, W_proj, b_proj):
    past_target = np.ascontiguousarray(past_target, np.float32)
    past_observed_values = np.ascontiguousarray(past_observed_values, np.float32)

    consts = _fold_weights(W_backbone, b_backbone, Wq, Memory, episodic_memory,
                           W_end, b_end, W_proj, b_proj)

    if "nc" not in _PROGRAM_CACHE:
        nc = _build_program()
        nc.finalize()
        _PROGRAM_CACHE["nc"] = nc
    nc = _PROGRAM_CACHE["nc"]

    in_maps = []
    for c in range(N_CORES):
        sl = slice(c * ROWS, (c + 1) * ROWS)
        m = {"xw": np.ascontiguousarray(
            np.concatenate([past_target[sl], past_observed_values[sl]], axis=1))}
        m.update(consts)
        in_maps.append(m)

    res = run_bass_kernel_spmd(nc, in_maps, list(range(N_CORES)))
    globals()["_LAST_RESULTS"] = res
    raw = np.concatenate([r["raw"] for r in res.results], axis=0)       # [B,72]
    # scl is [128, 8] per core with scale[p, t] = row t*128+p
    scale = np.concatenate(
        [r["scl"].T.reshape(ROWS, 1) for r in res.results], axis=0)     # [B,1]

    distr = raw.reshape(B_TOTAL, PRED_LEN, 3)
    loc = np.zeros_like(scale)
    return distr, loc, scale


# revision 42
# speedup vs baseline: 1.0479x; 1.0139x over previous
"""Trainium2 Bass kernel for nn_BimModel (retrieval_knn).

Strategy:
  - Algebraic folding (host, fp64): the final projection W_proj [64,3] applied
    per 64-wide block of y commutes with W_end, so W_end [1536,4608] folds to
    Wfold [4608,72].  All pure-linear chains fold similarly:
        logits = featA @ PM,  PM = WbA @ Wq @ Memory.T / sqrt(1536)   [98,512]
        sim*|h| = featA @ Pm, Pm = WbA @ mn.T                          [98,512]
        |h|^2   = rowsum((featA @ G) * featA), G = WbA @ WbA.T         [98,98]
        raw     = att @ MA + w_ep @ EC + featA @ FB (+ biases folded)
    featA = [scaled(96), log(scale), 1.0] (98-dim; the always-zero loc feature
    is dropped; biases ride on the const-1 column).
  - Top-8 episodic retrieval: done post-exp with nc.vector.max (one-instruction
    top-8 per partition); the gather becomes a masked-softmax matmul.
  - Data parallel over 8 NeuronCores: 1024 batch rows each; folded weights
    (~0.6 MB) replicated.
  - The attention-logits matmul runs in float32r (fast PE fp32 mode, ~1.6e-4
    rel err) — safe because softmax is smooth.  The similarity matmul (Pm)
    stays full fp32: top-8 SELECTION is discrete and sensitive to ties.
  - Mean-scaler + per-row scalars are batched [128, 8] across the 8 row-tiles;
    SBUF-only elementwise ops are offloaded to the otherwise-idle GPSIMD.
"""

import numpy as np

import concourse.bacc as bacc
import concourse.mybir as mybir
from concourse.tile import TileContext
from concourse.bass_utils import run_bass_kernel_spmd

N_CORES = 8
B_TOTAL = 8192
ROWS = B_TOTAL // N_CORES          # 1024 rows per core
N_TILES = ROWS // 128              # 8 tiles of 128 rows
C_IN = 96
MEM = 512
KF = 98                            # folded feature dim (scaled96 + logscale + one)
KP = 128                           # padded contraction dim
OUTW = 72                          # 24 preds x 3 params
PRED_LEN = 24
E2 = float(np.exp(2.0))            # ln(E2*x + E2) = softplus(ln x) + 2 trick

FP32 = mybir.dt.float32
F32R = mybir.dt.float32r
AX = mybir.AxisListType.X
OP = mybir.AluOpType
AF = mybir.ActivationFunctionType

_PROGRAM_CACHE: dict = {}


def _fold_weights(W_backbone, b_backbone, Wq, Memory, episodic_memory,
                  W_end, b_end, W_proj, b_proj):
    f64 = np.float64
    Wb = W_backbone.astype(f64)
    bb = b_backbone.astype(f64)
    Wqd = Wq.astype(f64)
    M = Memory.astype(f64)
    E = episodic_memory.astype(f64)
    We = W_end.astype(f64)
    be = b_end.astype(f64)
    Wp = W_proj.astype(f64)
    bp = b_proj.astype(f64)

    # 98-dim augmented backbone (drop always-zero loc feature, add bias row)
    WbA = np.concatenate([Wb[0:96], Wb[97:98], bb[None, :]], axis=0)     # [98,1536]

    Wfold = (We.T.reshape(4608, PRED_LEN, 64) @ Wp).reshape(4608, OUTW)  # [4608,72]
    bfold = (be.reshape(PRED_LEN, 64) @ Wp + bp).reshape(OUTW)           # [72]
    WfA, WfB, WfC = Wfold[0:1536], Wfold[1536:3072], Wfold[3072:4608]

    PM = WbA @ Wqd @ M.T / np.sqrt(f64(1536))                            # [98,512]
    En = E / np.clip(np.linalg.norm(E, axis=-1, keepdims=True), 1e-6, None)
    Pm = WbA @ En.T                                                      # [98,512]
    G = WbA @ WbA.T                                                      # [98,98]
    MA = M @ WfA                                                         # [512,72]
    EC = E @ WfC                                                         # [512,72]
    FB = WbA @ WfB                                                       # [98,72]
    FB[97] += bfold

    def pad_k(a):  # pad leading (contraction) dim 98 -> 128 with zeros
        out = np.zeros((KP, a.shape[1]), np.float32)
        out[: a.shape[0]] = a
        return out

    def chunked(a):  # [512,72] -> [128, 4*72] with chunk c at cols [72c:72c+72]
        return np.ascontiguousarray(
            a.reshape(4, 128, OUTW).transpose(1, 0, 2).reshape(128, 4 * OUTW),
            np.float32)

    GFB = np.concatenate([G, FB], axis=1)                                # [98,170]
    return {
        "PM": pad_k(PM),
        "Pm": pad_k(Pm),
        "GFB": pad_k(GFB),
        "MA": chunked(MA),
        "EC": chunked(EC),
        "ident": np.eye(128, dtype=np.float32),
    }


def _build_program(gp_offload=True, ft_own_pool=True, psl=1, psh=1, pst=3, wpb=6, spb=4):
    nc = bacc.Bacc()

    xw_d = nc.dram_tensor("xw", [ROWS, 2 * C_IN], FP32, kind="ExternalInput")
    pm_d = nc.dram_tensor("PM", [KP, MEM], F32R, kind="ExternalInput")
    pmn_d = nc.dram_tensor("Pm", [KP, MEM], FP32, kind="ExternalInput")
    gfb_d = nc.dram_tensor("GFB", [KP, KF + OUTW], FP32, kind="ExternalInput")
    ma_d = nc.dram_tensor("MA", [128, 4 * OUTW], FP32, kind="ExternalInput")
    ec_d = nc.dram_tensor("EC", [128, 4 * OUTW], FP32, kind="ExternalInput")
    id_d = nc.dram_tensor("ident", [128, 128], FP32, kind="ExternalInput")

    raw_d = nc.dram_tensor("raw", [ROWS, OUTW], FP32, kind="ExternalOutput")
    scl_d = nc.dram_tensor("scl", [128, N_TILES], FP32, kind="ExternalOutput")

    with TileContext(nc) as tc:
        with (
            tc.tile_pool(name="consts", bufs=1) as cpool,
            tc.tile_pool(name="resid", bufs=1) as rp,
            tc.tile_pool(name="work", bufs=wpb) as wp,
            tc.tile_pool(name="small", bufs=spb) as sp,
            tc.tile_pool(name="psL", bufs=psl, space="PSUM") as psL,       # logits
            tc.tile_pool(name="psH", bufs=psh, space="PSUM") as psH,                           # hm
            tc.tile_pool(name="psG", bufs=1, space="PSUM") as psG,       # gfb
            tc.tile_pool(name="psT", bufs=pst, space="PSUM") as psT,       # transposes
            tc.tile_pool(name="psO", bufs=1, space="PSUM") as psO,       # out72
            tc.tile_pool(name="psF", bufs=1, space="PSUM") as psF,       # featT
        ):
            ft_pool = psF if ft_own_pool else psT
            ft_tag = "fT" if ft_own_pool else "tr"
            gv = nc.gpsimd if gp_offload else nc.vector
            pm_sb = cpool.tile_from(pm_d[:])
            pmn_sb = cpool.tile_from(pmn_d[:])
            gfb_sb = cpool.tile_from(gfb_d[:])
            ma_sb = cpool.tile_from(ma_d[:])
            ec_sb = cpool.tile_from(ec_d[:])
            id_sb = cpool.tile_from(id_d[:])
            e2c = cpool.tile([128, 1], FP32, tag="e2c")
            nc.vector.memset(e2c[:], E2)

            # Pin the ACT table to natural_log_exp_and_others (covers Exp, Ln,
            # Copy, Abs) so bacc's per-function chooser doesn't thrash between
            # exp_and_others and natural_log (45 reloads ~= 58us otherwise).
            nc.scalar.add_instruction(mybir.InstLoadActFuncSet(
                name=nc.get_next_instruction_name(), act_func_set_id=6,
                ins=[], outs=[]))

            # ---- load all row-tiles; batched mean-scaler over [128, 8] ----
            xin = rp.tile([128, N_TILES * 2 * C_IN], FP32, tag="xin")
            xin3 = xin[:].rearrange("p (t c) -> p t c", c=192)
            nc.sync.dma_start(out=xin3,
                              in_=xw_d[:].rearrange("(t p) c -> p t c", p=128))
            xabs = rp.tile([128, N_TILES * C_IN], FP32, tag="xabs")
            xabs3 = xabs[:].rearrange("p (t c) -> p t c", c=C_IN)
            nc.vector.tensor_tensor(out=xabs3, in0=xin3[:, :, 0:C_IN],
                                    in1=xin3[:, :, C_IN:192], op=OP.mult)
            ts8 = rp.tile([128, N_TILES], FP32, tag="ts8")
            nc.vector.tensor_reduce(out=ts8[:], in_=xabs3, axis=AX, op=OP.add,
                                    apply_absolute_value=True)
            nobs8 = rp.tile([128, N_TILES], FP32, tag="nobs8")
            nc.vector.tensor_reduce(out=nobs8[:], in_=xin3[:, :, C_IN:192],
                                    axis=AX, op=OP.add)
            gv.tensor_scalar_max(nobs8[:], nobs8[:], 1.0)
            rn8 = rp.tile([128, N_TILES], FP32, tag="rn8")
            nc.vector.reciprocal(out=rn8[:], in_=nobs8[:])
            scale8 = rp.tile([128, N_TILES], FP32, tag="scale8")
            nc.vector.tensor_tensor(out=scale8[:], in0=ts8[:], in1=rn8[:],
                                    op=OP.mult)
            nc.vector.tensor_scalar_max(scale8[:], scale8[:], 1e-10)
            nc.sync.dma_start(out=scl_d[:], in_=scale8[:])
            logscale8 = rp.tile([128, N_TILES], FP32, tag="logscale8")
            nc.scalar.activation(out=logscale8[:], in_=scale8[:], func=AF.Ln)
            rs8 = rp.tile([128, N_TILES], FP32, tag="rs8")
            nc.vector.reciprocal(out=rs8[:], in_=scale8[:])

            for t in range(N_TILES):
                rs0 = t * 128
                x_sb = xin[:, t * 192:t * 192 + C_IN]

                # ---- featA = [x/scale, ln(scale), 1.0] ----
                featA = wp.tile([128, KF], FP32, tag="featA")
                nc.vector.tensor_scalar(out=featA[:, 0:C_IN], in0=x_sb,
                                        scalar1=rs8[:, t:t + 1], scalar2=None,
                                        op0=OP.mult)
                gv.tensor_copy(out=featA[:, 96:97],
                                      in_=logscale8[:, t:t + 1])
                gv.memset(featA[:, 97:98], 1.0)

                # ---- transpose featA -> featT[0:98]; pad rows are garbage,
                #      harmless: weight rows 98:127 are zero ----
                fT_ps = ft_pool.tile([128, 128], FP32, tag=ft_tag)
                nc.tensor.transpose(fT_ps[0:KF, :], featA[:], id_sb[:])
                featT = wp.tile([128, 128], FP32, tag="featT")
                gv.memset(featT[96:128, :], 0.0)
                nc.scalar.copy(out=featT[0:KF, :], in_=fT_ps[0:KF, :])
                featTr = wp.tile([128, 128], F32R, tag="featTr")
                nc.vector.tensor_copy(out=featTr[0:KF, :], in_=fT_ps[0:KF, :])

                # ---- stage-1 matmuls (featT stationary) ----
                logits_ps = psL.tile([128, MEM], FP32, tag="logits")
                hm_ps = psH.tile([128, MEM], FP32, tag="hm")
                gfb_ps = psG.tile([128, KF], FP32, tag="gfbo")
                nc.tensor.matmul(logits_ps[:], featTr[0:KF, :], pm_sb[0:KF, :],
                                 start=True, stop=True)
                nc.tensor.matmul(hm_ps[:], featT[:], pmn_sb[:], start=True,
                                 stop=True)
                nc.tensor.matmul(gfb_ps[:], featT[:], gfb_sb[:, 0:KF],
                                 start=True, stop=True)

                # ---- attention softmax (no max-sub; logits bounded) ----
                e_att = wp.tile([128, MEM], FP32, tag="e_att")
                datt = sp.tile([128, 1], FP32, tag="datt")
                nc.scalar.activation(out=e_att[:], in_=logits_ps[:], func=AF.Exp,
                                     accum_out=datt[:])
                rdatt = sp.tile([128, 1], FP32, tag="rdatt")
                nc.vector.reciprocal(out=rdatt[:], in_=datt[:])
                e_attn = wp.tile([128, MEM], FP32, tag="e_attn")
                gv.tensor_scalar(out=e_attn[:], in0=e_att[:],
                                        scalar1=rdatt[:], scalar2=None,
                                        op0=OP.mult)

                # ---- 1/||h|| via featA G featA^T; rsqrt = exp(-0.5 ln) ----
                hgf = wp.tile([128, KF], FP32, tag="hgf")
                nc.vector.tensor_tensor(out=hgf[:], in0=gfb_ps[:, 0:KF],
                                        in1=featA[:], op=OP.mult)
                normsq = sp.tile([128, 1], FP32, tag="normsq")
                nc.vector.tensor_reduce(out=normsq[:], in_=hgf[:], axis=AX,
                                        op=OP.add)
                nc.vector.tensor_scalar_max(normsq[:], normsq[:], 1e-12)
                lnq = sp.tile([128, 1], FP32, tag="lnq")
                nc.scalar.activation(out=lnq[:], in_=normsq[:], func=AF.Ln)
                r_sb = sp.tile([128, 1], FP32, tag="r")
                nc.scalar.activation(out=r_sb[:], in_=lnq[:], func=AF.Exp,
                                     scale=-0.5)

                # ---- episodic: top-8 of cos post-exp, masked softmax ----
                e_cos = wp.tile([128, MEM], FP32, tag="e_cos")
                nc.scalar.activation(out=e_cos[:], in_=hm_ps[:], func=AF.Exp,
                                     scale=r_sb[:])
                m8 = sp.tile([128, 8], FP32, tag="m8")
                nc.vector.max(out=m8[:], in_=e_cos[:])
                dep = sp.tile([128, 1], FP32, tag="dep")
                nc.vector.tensor_reduce(out=dep[:], in_=m8[:], axis=AX, op=OP.add)
                rdep = sp.tile([128, 1], FP32, tag="rdep")
                nc.vector.reciprocal(out=rdep[:], in_=dep[:])
                mask_s = wp.tile([128, MEM], FP32, tag="mask")
                gv.tensor_scalar(out=mask_s[:], in0=e_cos[:],
                                        scalar1=m8[:, 7:8], scalar2=rdep[:],
                                        op0=OP.is_ge, op1=OP.mult)
                e_ep = wp.tile([128, MEM], FP32, tag="e_ep")
                nc.vector.tensor_tensor(out=e_ep[:], in0=e_cos[:], in1=mask_s[:],
                                        op=OP.mult)

                # ---- transpose e_attn / e_ep via PE, copy PSUM->SBUF ----
                eT = wp.tile([128, MEM], FP32, tag="eT")
                epT = wp.tile([128, MEM], FP32, tag="epT")
                for c in range(4):
                    sl = slice(c * 128, (c + 1) * 128)
                    tr1 = psT.tile([128, 128], FP32, tag="tr")
                    nc.tensor.transpose(tr1[:], e_attn[:, sl], id_sb[:])
                    if c % 2 == 0:
                        nc.vector.tensor_copy(out=eT[:, sl], in_=tr1[:])
                    else:
                        nc.scalar.copy(out=eT[:, sl], in_=tr1[:])
                    tr2 = psT.tile([128, 128], FP32, tag="tr")
                    nc.tensor.transpose(tr2[:], e_ep[:, sl], id_sb[:])
                    if c % 2 == 0:
                        nc.scalar.copy(out=epT[:, sl], in_=tr2[:])
                    else:
                        nc.vector.tensor_copy(out=epT[:, sl], in_=tr2[:])

                # ---- stage-2: raw72 = feat@FB + att@MA + ep@EC (PSUM accum).
                #      Tile pairs share one [128,144] PSUM tile so the head
                #      runs once per pair (param stride 3 is uniform across
                #      both 72-col halves). ----
                if t % 2 == 0:
                    o72_pair = psO.tile([128, 2 * OUTW], FP32, tag="o72")
                o72_ps = o72_pair[:, (t % 2) * OUTW:(t % 2 + 1) * OUTW]
                nc.tensor.matmul(o72_ps, featT[:], gfb_sb[:, KF:KF + OUTW],
                                 start=True, stop=False)
                for c in range(4):
                    sl = slice(c * 128, (c + 1) * 128)
                    cw = slice(c * OUTW, (c + 1) * OUTW)
                    nc.tensor.matmul(o72_ps, eT[:, sl], ma_sb[:, cw],
                                     start=False, stop=False)
                for c in range(4):
                    sl = slice(c * 128, (c + 1) * 128)
                    cw = slice(c * OUTW, (c + 1) * OUTW)
                    nc.tensor.matmul(o72_ps, epT[:, sl], ec_sb[:, cw],
                                     start=False, stop=(c == 3))
                if t % 2 == 0:
                    continue

                # ---- StudentT head for the pair (reads [128,144] PSUM) ----
                out_sb = wp.tile([128, 2 * OUTW], FP32, tag="out")
                t48 = wp.tile([128, 2 * PRED_LEN], FP32, tag="t48")
                # df = 2 + softplus(raw0) = ln(E2*exp(raw0) + E2)
                nc.scalar.activation(out=t48[:], in_=o72_pair[:, 0::3], func=AF.Exp)
                nc.scalar.activation(out=out_sb[:, 0::3], in_=t48[:], func=AF.Ln,
                                     scale=E2, bias=e2c[:])
                nc.vector.tensor_copy(out=out_sb[:, 1::3], in_=o72_pair[:, 1::3])
                t48b = wp.tile([128, 2 * PRED_LEN], FP32, tag="t48b")
                nc.scalar.activation(out=t48b[:], in_=o72_pair[:, 2::3], func=AF.Exp)
                nc.scalar.activation(out=out_sb[:, 2::3], in_=t48b[:], func=AF.Ln,
                                     bias=1.0)
                nc.sync.dma_start(
                    out=raw_d[rs0 - 128:rs0 + 128, :].rearrange(
                        "(u p) w -> p u w", p=128),
                    in_=out_sb[:].rearrange("p (u w) -> p u w", w=OUTW))

    return nc


def kernel(past_target, past_observed_values, W_backbone, b_backbone, Wq,
           Memory, episodic_memory, W_end, b_end, W_proj, b_proj):
    past_target = np.ascontiguousarray(past_target, np.float32)
    past_observed_values = np.ascontiguousarray(past_observed_values, np.float32)

    consts = _fold_weights(W_backbone, b_backbone, Wq, Memory, episodic_memory,
                           W_end, b_end, W_proj, b_proj)

    if "nc" not in _PROGRAM_CACHE:
        nc = _build_program()
        nc.finalize()
        _PROGRAM_CACHE["nc"] = nc
    nc = _PROGRAM_CACHE["nc"]

    in_maps = []
    for c in range(N_CORES):
        sl = slice(c * ROWS, (c + 1) * ROWS)
        m = {"xw": np.ascontiguousarray(
            np.concatenate([past_target[sl], past_observed_values[sl]], axis=1))}
        m.update(consts)
        in_maps.append(m)

    res = run_bass_kernel_spmd(nc, in_maps, list(range(N_CORES)))
    globals()["_LAST_RESULTS"] = res
    raw = np.concatenate([r["raw"] for r in res.results], axis=0)       # [B,72]
    # scl is [128, 8] per core with scale[p, t] = row t*128+p
    scale = np.concatenate(
        [r["scl"].T.reshape(ROWS, 1) for r in res.results], axis=0)     # [B,1]

    distr = raw.reshape(B_TOTAL, PRED_LEN, 3)
    loc = np.zeros_like(scale)
    return distr, loc, scale
